# revision 1
# baseline (speedup 1.0000x reference)
"""DenseCaps routing kernel for 8x Trainium2 NeuronCores.

Shapes (hardcoded): inputs (16, 2048, 16) f32, w (2048, 16, 64, 32) f32.
Sharding: ch_i (2048) split 8 ways -> 256 i's per core. Each core computes
u[b, i_loc, j, m] via a block-diagonal stationary matmul streaming its w
shard once, keeps u resident in SBUF (bf16), runs the 3-iteration dynamic
routing locally, and AllReduces the tiny s[b, j, m] partial sums.
Output v (16, 64, 32) f32 is identical on all cores; core 0's is returned.
"""

import sys
from contextlib import ExitStack

import numpy as np

sys.path.insert(0, "/opt/trn_rl_repo")

import concourse.bass as bass
import concourse.bacc as bacc
import concourse.tile as tile
from concourse import mybir
from concourse.bass_utils import run_bass_kernel_spmd

F32 = mybir.dt.float32
BF16 = mybir.dt.bfloat16

B = 16
CH_I = 2048
N_I = 16
CH_J = 64
N_J = 32
JM = CH_J * N_J  # 2048
N_CORES = 8
I_LOC = CH_I // N_CORES  # 256
N_T = I_LOC // 8  # 32 production tiles, 8 i's each
EPS = 1e-7

_CACHE = {}


def _build_program(trace=False):
    nc = bacc.Bacc("TRN2", target_bir_lowering=False, debug=False,
                   num_devices=N_CORES)

    xblk_d = nc.dram_tensor("xblk", [N_T, 128, 128], F32, kind="ExternalInput")
    w_d = nc.dram_tensor("w", [I_LOC * N_I, JM], F32, kind="ExternalInput")
    s16_d = nc.dram_tensor("s16", [128, 16], BF16, kind="ExternalInput")
    r8_d = nc.dram_tensor("r8", [16, 128], F32, kind="ExternalInput")
    v_d = nc.dram_tensor("v", [B, JM], F32, kind="ExternalOutput")

    cc_in = [nc.dram_tensor(f"cc_in{r}", [B, JM], F32) for r in range(3)]
    cc_out = [nc.dram_tensor(f"cc_out{r}", [B, JM], F32, addr_space="Shared")
              for r in range(3)]

    with tile.TileContext(nc) as tc, ExitStack() as ctx:
        _kernel_body(ctx, tc, xblk_d, w_d, s16_d, r8_d, v_d, cc_in, cc_out)
    nc.compile()
    return nc


def _kernel_body(ctx, tc, xblk_d, w_d, s16_d, r8_d, v_d, cc_in, cc_out):
    nc = tc.nc
    Act = mybir.ActivationFunctionType
    Alu = mybir.AluOpType
    groups = [list(range(N_CORES))]

    const_pool = ctx.enter_context(tc.tile_pool(name="consts", bufs=1))
    s16 = const_pool.tile([128, 16], BF16)
    nc.sync.dma_start(s16[:], s16_d[:])
    r8 = const_pool.tile([16, 128], F32)
    nc.sync.dma_start(r8[:], r8_d[:])
    eps_t = const_pool.tile([16, 1], F32)
    nc.vector.memset(eps_t[:], EPS)

    u_pool = ctx.enter_context(tc.tile_pool(name="u", bufs=N_T))
    u_tiles = []

    # small persistent buffers
    sv_pool = ctx.enter_context(tc.tile_pool(name="sv", bufs=1))
    a1_pool = ctx.enter_context(tc.tile_pool(name="a1", bufs=N_T))
    a1_tiles = []

    # ---------------- Phase A: produce u, accumulate s0 ----------------
    with tc.tile_pool(name="wbuf", bufs=2) as w_pool, \
         tc.tile_pool(name="xbuf", bufs=2) as x_pool, \
         tc.tile_pool(name="uprod", bufs=2, space="PSUM") as up_pool, \
         tc.tile_pool(name="s0ps", bufs=1, space="PSUM") as s0_pool:
        s0_ps = s0_pool.tile([16, JM], F32)
        for t in range(N_T):
            wt = w_pool.tile([128, JM], F32)
            nc.sync.dma_start(wt[:], w_d[t * 128:(t + 1) * 128, :])
            xt = x_pool.tile([128, 128], F32)
            nc.sync.dma_start(xt[:], xblk_d[t])
            ut = u_pool.tile([128, JM], BF16)
            u_tiles.append(ut)
            for h in range(2):
                up = up_pool.tile([128, 1024], F32)
                for q in range(2):
                    nc.tensor.matmul(
                        up[:, q * 512:(q + 1) * 512], xt[:],
                        wt[:, h * 1024 + q * 512: h * 1024 + (q + 1) * 512],
                        start=True, stop=True)
                dst = ut[:, h * 1024:(h + 1) * 1024]
                if h == 0:
                    nc.scalar.activation(dst, up[:], Act.Copy)
                else:
                    nc.vector.tensor_copy(dst, up[:])
                for q in range(2):
                    o0 = h * 1024 + q * 512
                    nc.tensor.matmul(
                        s0_ps[:, o0:o0 + 512], s16[:],
                        ut[:, o0:o0 + 512],
                        start=(t == 0), stop=(t == N_T - 1))
        s_sb = sv_pool.tile([16, JM], F32, tag="s_sb")
        nc.scalar.activation(s_sb[:, :1024], s0_ps[:, :1024], Act.Copy)
        nc.vector.tensor_copy(s_sb[:, 1024:], s0_ps[:, 1024:])

    def allreduce_squash(r, s_sb):
        """AllReduce partial s, then squash -> v_sb (f32 (16, JM))."""
        nc.sync.dma_start(cc_in[r][:], s_sb[:])
        nc.gpsimd.collective_compute(
            "AllReduce", Alu.add, replica_groups=groups,
            ins=[cc_in[r][:]], outs=[cc_out[r][:]])
        sf = sv_pool.tile([16, JM], F32, tag="sf")
        nc.sync.dma_start(sf[:], cc_out[r][:])
        t2 = sv_pool.tile([16, JM], F32, tag="t2")
        nc.vector.tensor_mul(t2[:], sf[:], sf[:])
        sq = sv_pool.tile([16, CH_J], F32, tag="sq")
        nc.vector.tensor_reduce(
            sq[:], t2[:].rearrange("p (j m) -> p j m", m=N_J),
            axis=mybir.AxisListType.X, op=Alu.add)
        rt = sv_pool.tile([16, CH_J], F32, tag="rt")
        nc.scalar.activation(rt[:], sq[:], Act.Sqrt, bias=eps_t[:])
        onep = sv_pool.tile([16, CH_J], F32, tag="onep")
        nc.vector.tensor_scalar_add(onep[:], sq[:], 1.0)
        den = sv_pool.tile([16, CH_J], F32, tag="den")
        nc.vector.tensor_mul(den[:], rt[:], onep[:])
        rec = sv_pool.tile([16, CH_J], F32, tag="rec")
        nc.vector.reciprocal(rec[:], den[:])
        fs = sv_pool.tile([16, CH_J], F32, tag="fs")
        nc.vector.tensor_mul(fs[:], sq[:], rec[:])
        vv = sv_pool.tile([16, JM], F32, tag="vv")
        nc.vector.tensor_mul(
            vv[:].rearrange("p (j m) -> p j m", m=N_J),
            sf[:].rearrange("p (j m) -> p j m", m=N_J),
            fs[:].unsqueeze(-1).broadcast_to((16, CH_J, N_J)))
        return vv

    v_sb = allreduce_squash(0, s_sb)

    # ---------------- Phases C: routing passes r=1,2 ----------------
    for r in (1, 2):
        with tc.tile_pool(name=f"vr{r}", bufs=1) as vr_pool, \
             tc.tile_pool(name=f"scr{r}", bufs=2) as scr_pool, \
             tc.tile_pool(name=f"sm{r}", bufs=4) as sm_pool, \
             tc.tile_pool(name=f"vrps{r}", bufs=2, space="PSUM") as vr_ps_pool, \
             tc.tile_pool(name=f"sps{r}", bufs=1, space="PSUM") as s_ps_pool:
            # v_rep (128, JM) bf16: broadcast v over the 8 i-slots
            v_rep = vr_pool.tile([128, JM], BF16)
            for q in range(4):
                vp = vr_ps_pool.tile([128, 512], F32)
                nc.tensor.matmul(vp[:], r8[:],
                                 v_sb[:, q * 512:(q + 1) * 512],
                                 start=True, stop=True)
                if q % 2 == 0:
                    nc.scalar.activation(v_rep[:, q * 512:(q + 1) * 512],
                                         vp[:], Act.Copy)
                else:
                    nc.vector.tensor_copy(v_rep[:, q * 512:(q + 1) * 512],
                                          vp[:])
            s_ps = s_ps_pool.tile([16, JM], F32)
            for t in range(N_T):
                ut = u_tiles[t]
                uv = scr_pool.tile([128, JM], BF16, tag="uv")
                nc.vector.tensor_mul(uv[:], ut[:], v_rep[:])
                if r == 1:
                    a1 = a1_pool.tile([128, CH_J], F32)
                    a1_tiles.append(a1)
                    nc.vector.tensor_reduce(
                        a1[:], uv[:].rearrange("p (j m) -> p j m", m=N_J),
                        axis=mybir.AxisListType.X, op=Alu.add)
                    logits = a1
                else:
                    a2 = sm_pool.tile([128, CH_J], F32, tag="a2")
                    nc.vector.tensor_reduce(
                        a2[:], uv[:].rearrange("p (j m) -> p j m", m=N_J),
                        axis=mybir.AxisListType.X, op=Alu.add)
                    lg = sm_pool.tile([128, CH_J], F32, tag="lg")
                    nc.vector.tensor_add(lg[:], a2[:], a1_tiles[t][:])
                    logits = lg
                expt = sm_pool.tile([128, CH_J], F32, tag="expt")
                se = sm_pool.tile([128, 1], F32, tag="se")
                nc.scalar.activation(expt[:], logits[:], Act.Exp,
                                     scale=8.0, accum_out=se[:])
                se2 = sm_pool.tile([128, 1], F32, tag="se2")
                nc.vector.tensor_scalar_mul(se2[:], se[:], 1.0 / CH_J)
                rs = sm_pool.tile([128, 1], F32, tag="rs")
                nc.vector.reciprocal(rs[:], se2[:])
                ct = sm_pool.tile([128, CH_J], BF16, tag="ct")
                nc.vector.tensor_scalar_mul(ct[:], expt[:], rs[:])
                cexp = scr_pool.tile([128, JM], BF16, tag="cexp")
                nc.scalar.activation(
                    cexp[:].rearrange("p (j m) -> p j m", m=N_J),
                    ct[:].unsqueeze(-1).broadcast_to((128, CH_J, N_J)),
                    Act.Copy)
                cu = scr_pool.tile([128, JM], BF16, tag="cu")
                nc.vector.tensor_mul(cu[:], ut[:], cexp[:])
                for q in range(4):
                    nc.tensor.matmul(
                        s_ps[:, q * 512:(q + 1) * 512], s16[:],
                        cu[:, q * 512:(q + 1) * 512],
                        start=(t == 0), stop=(t == N_T - 1))
            s_sb2 = sv_pool.tile([16, JM], F32, tag="s_sb")
            nc.scalar.activation(s_sb2[:, :1024], s_ps[:, :1024], Act.Copy)
            nc.vector.tensor_copy(s_sb2[:, 1024:], s_ps[:, 1024:])
        v_sb = allreduce_squash(r, s_sb2)

    nc.sync.dma_start(v_d[:], v_sb[:])


def _host_inputs(inputs, w):
    """Build per-core input maps (host-side shard + block-diag pack)."""
    x = np.asarray(inputs, dtype=np.float32)
    w = np.asarray(w, dtype=np.float32)
    s16 = np.zeros((128, 16), dtype=np.float32)
    for i8 in range(8):
        for b in range(16):
            s16[i8 * 16 + b, b] = 1.0
    import ml_dtypes
    s16 = s16.astype(ml_dtypes.bfloat16)
    r8 = np.zeros((16, 128), dtype=np.float32)
    for i8 in range(8):
        for b in range(16):
            r8[b, i8 * 16 + b] = 1.0
    in_maps = []
    for k in range(N_CORES):
        i0 = k * I_LOC
        wk = w[i0:i0 + I_LOC].reshape(I_LOC * N_I, JM).copy()
        xk = x[:, i0:i0 + I_LOC, :]  # (B, 256, 16)
        xblk = np.zeros((N_T, 128, 128), dtype=np.float32)
        # xblk[t, i8*16+n, i8*16+b] = x[b, i0+8t+i8, n]
        xv = xk.transpose(1, 2, 0).reshape(N_T, 8, N_I, B)  # (t, i8, n, b)
        for i8 in range(8):
            xblk[:, i8 * 16:i8 * 16 + N_I, i8 * 16:i8 * 16 + B] = xv[:, i8]
        in_maps.append({"xblk": xblk, "w": wk, "s16": s16, "r8": r8})
    return in_maps


def kernel(inputs, w, _trace=False):
    key = "nc"
    if key not in _CACHE:
        _CACHE[key] = _build_program()
    nc = _CACHE[key]
    in_maps = _host_inputs(inputs, w)
    res = run_bass_kernel_spmd(nc, in_maps, list(range(N_CORES)),
                               trace=_trace)
    v = res.results[0]["v"].reshape(B, CH_J, N_J).astype(np.float32)
    if _trace:
        kernel._last = res
    return v



# revision 8
# speedup vs baseline: 1.2757x; 1.2757x over previous
"""DenseCaps routing kernel for 8x Trainium2 NeuronCores.

Shapes (hardcoded): inputs (16, 2048, 16) f32, w (2048, 16, 64, 32) f32.
Sharding: ch_i (2048) split 8 ways -> 256 i's per core. Each core computes
u[b, i_loc, j, m] via a block-diagonal stationary matmul streaming its w
shard once (bf16), keeps u resident in SBUF (bf16), runs the 3-iteration
dynamic routing locally, and AllReduces the tiny s[b, j, m] partial sums.
Output v (16, 64, 32) f32 is identical on all cores; core 0's is returned.
"""

import sys
from contextlib import ExitStack

import numpy as np

sys.path.insert(0, "/opt/trn_rl_repo")

import concourse.bass as bass
import concourse.bacc as bacc
import concourse.tile as tile
from concourse import mybir
from concourse.bass_utils import run_bass_kernel_spmd

F32 = mybir.dt.float32
BF16 = mybir.dt.bfloat16

B = 16
CH_I = 2048
N_I = 16
CH_J = 64
N_J = 32
JM = CH_J * N_J  # 2048
N_CORES = 8
I_LOC = CH_I // N_CORES  # 256
N_T = I_LOC // 8  # 32 production tiles, 8 i's each
EPS = 1e-7

_CACHE = {}


def _build_program(trace=False):
    nc = bacc.Bacc("TRN2", target_bir_lowering=False, debug=False,
                   num_devices=N_CORES)

    xblk_d = nc.dram_tensor("xblk", [N_T, 128, 128], BF16, kind="ExternalInput")
    xsum_d = nc.dram_tensor("xsum", [N_T, 128, 16], BF16, kind="ExternalInput")
    w_d = nc.dram_tensor("w", [I_LOC * N_I, JM], BF16, kind="ExternalInput")
    s16_d = nc.dram_tensor("s16", [128, 16], BF16, kind="ExternalInput")
    r8_d = nc.dram_tensor("r8", [16, 128], BF16, kind="ExternalInput")
    v_d = nc.dram_tensor("v", [B, JM], F32, kind="ExternalOutput")

    cc_in = [nc.dram_tensor(f"cc_in{r}", [B, JM], F32) for r in range(3)]
    cc_out = [nc.dram_tensor(f"cc_out{r}", [B, JM], F32, addr_space="Shared")
              for r in range(3)]

    with tile.TileContext(nc) as tc, ExitStack() as ctx:
        _kernel_body(ctx, tc, xblk_d, xsum_d, w_d, s16_d, r8_d, v_d,
                     cc_in, cc_out)
    nc.compile()
    return nc


def _kernel_body(ctx, tc, xblk_d, xsum_d, w_d, s16_d, r8_d, v_d,
                 cc_in, cc_out):
    nc = tc.nc
    Act = mybir.ActivationFunctionType
    Alu = mybir.AluOpType
    groups = [list(range(N_CORES))]

    const_pool = ctx.enter_context(tc.tile_pool(name="consts", bufs=1))
    s16 = const_pool.tile([128, 16], BF16)
    nc.sync.dma_start(s16[:], s16_d[:])
    r8 = const_pool.tile([16, 128], BF16)
    nc.sync.dma_start(r8[:], r8_d[:])
    eps_t = const_pool.tile([16, 1], F32)
    nc.vector.memset(eps_t[:], EPS)

    u_pool = ctx.enter_context(tc.tile_pool(name="u", bufs=N_T))
    u_tiles = []

    # small persistent buffers
    sv_pool = ctx.enter_context(tc.tile_pool(name="sv", bufs=1))
    a1_pool = ctx.enter_context(tc.tile_pool(name="a1", bufs=N_T))
    a1_tiles = []

    # ---------------- Phase A: produce u, accumulate s0 ----------------
    # s0 is accumulated straight from the w stream (stationary = per-tile
    # x summed over the 8 i-slots), so it does not depend on the u casts.
    with tc.tile_pool(name="wbuf", bufs=3) as w_pool, \
         tc.tile_pool(name="xbuf", bufs=3) as x_pool, \
         tc.tile_pool(name="xsbuf", bufs=3) as xs_pool, \
         tc.tile_pool(name="uprod", bufs=2, space="PSUM") as up_pool, \
         tc.tile_pool(name="s0ps", bufs=1, space="PSUM") as s0_pool:
        s0_ps = s0_pool.tile([16, JM], F32)
        for t in range(N_T):
            wt = w_pool.tile([128, JM], BF16)
            nc.sync.dma_start(wt[:], w_d[t * 128:(t + 1) * 128, :])
            xt = x_pool.tile([128, 128], BF16)
            nc.sync.dma_start(xt[:], xblk_d[t])
            xs = xs_pool.tile([128, 16], BF16)
            nc.sync.dma_start(xs[:], xsum_d[t])
            ut = u_pool.tile([128, JM], BF16)
            u_tiles.append(ut)
            for h in range(2):
                up = up_pool.tile([128, 1024], F32)
                for q in range(2):
                    o0 = h * 1024 + q * 512
                    nc.tensor.matmul(
                        up[:, q * 512:(q + 1) * 512], xt[:],
                        wt[:, o0:o0 + 512], start=True, stop=True)
                    nc.tensor.matmul(
                        s0_ps[:, o0:o0 + 512], xs[:], wt[:, o0:o0 + 512],
                        start=(t == 0), stop=(t == N_T - 1))
                dst = ut[:, h * 1024:(h + 1) * 1024]
                if h == 0:
                    nc.scalar.activation(dst, up[:], Act.Copy)
                else:
                    nc.vector.tensor_copy(dst, up[:])
        s_sb = sv_pool.tile([16, JM], F32, tag="s_sb")
        nc.scalar.activation(s_sb[:, :1024], s0_ps[:, :1024], Act.Copy)
        nc.vector.tensor_copy(s_sb[:, 1024:], s0_ps[:, 1024:])

    def allreduce_squash(r, s_sb, last):
        """AllReduce partial s, then squash -> v (bf16 for v_rep, or f32)."""
        nc.sync.dma_start(cc_in[r][:], s_sb[:])
        nc.gpsimd.collective_compute(
            "AllReduce", Alu.add, replica_groups=groups,
            ins=[cc_in[r][:]], outs=[cc_out[r][:]])
        sf = sv_pool.tile([16, JM], F32, tag="sf")
        nc.sync.dma_start(sf[:], cc_out[r][:])
        t2 = sv_pool.tile([16, JM], F32, tag="s_sb")
        nc.scalar.activation(t2[:], sf[:], Act.Square)
        sq = sv_pool.tile([16, CH_J], F32, tag="sq")
        nc.vector.tensor_reduce(
            sq[:], t2[:].rearrange("p (j m) -> p j m", m=N_J),
            axis=mybir.AxisListType.X, op=Alu.add)
        rt = sv_pool.tile([16, CH_J], F32, tag="rt")
        nc.scalar.activation(rt[:], sq[:], Act.Sqrt, bias=eps_t[:])
        onep = sv_pool.tile([16, CH_J], F32, tag="onep")
        nc.vector.tensor_scalar_add(onep[:], sq[:], 1.0)
        den = sv_pool.tile([16, CH_J], F32, tag="den")
        nc.vector.tensor_mul(den[:], rt[:], onep[:])
        rec = sv_pool.tile([16, CH_J], F32, tag="rec")
        nc.vector.reciprocal(rec[:], den[:])
        fs = sv_pool.tile([16, CH_J], F32, tag="fs")
        nc.vector.tensor_mul(fs[:], sq[:], rec[:])
        vv = sv_pool.tile([16, JM], F32 if last else BF16, tag="vv")
        nc.vector.tensor_mul(
            vv[:].rearrange("p (j m) -> p j m", m=N_J),
            sf[:].rearrange("p (j m) -> p j m", m=N_J),
            fs[:].unsqueeze(-1).broadcast_to((16, CH_J, N_J)))
        return vv

    v_sb = allreduce_squash(0, s_sb, last=False)

    # ---------------- Phases C: routing passes r=1,2 ----------------
    for r in (1, 2):
        with tc.tile_pool(name=f"vr{r}", bufs=1) as vr_pool, \
             tc.tile_pool(name=f"scr{r}", bufs=2) as scr_pool, \
             tc.tile_pool(name=f"ce{r}", bufs=2) as ce_pool, \
             tc.tile_pool(name=f"cu{r}", bufs=2) as cu_pool, \
             tc.tile_pool(name=f"t1_{r}", bufs=2) as t1_pool, \
             tc.tile_pool(name=f"t2_{r}", bufs=2) as t2_pool, \
             tc.tile_pool(name=f"t3_{r}", bufs=2) as t3_pool, \
             tc.tile_pool(name=f"sm{r}", bufs=3) as sm_pool, \
             tc.tile_pool(name=f"vrps{r}", bufs=2, space="PSUM") as vr_ps_pool, \
             tc.tile_pool(name=f"sps{r}", bufs=1, space="PSUM") as s_ps_pool:
            # v_rep (128, JM) bf16: broadcast v over the 8 i-slots
            v_rep = vr_pool.tile([128, JM], BF16)
            for h in range(2):
                vp = vr_ps_pool.tile([128, 1024], F32)
                for q in range(2):
                    nc.tensor.matmul(
                        vp[:, q * 512:(q + 1) * 512], r8[:],
                        v_sb[:, h * 1024 + q * 512:h * 1024 + (q + 1) * 512],
                        start=True, stop=True)
                if h == 0:
                    nc.scalar.activation(v_rep[:, h * 1024:(h + 1) * 1024],
                                         vp[:], Act.Copy)
                else:
                    nc.vector.tensor_copy(v_rep[:, h * 1024:(h + 1) * 1024],
                                          vp[:])
            s_ps = s_ps_pool.tile([16, JM], F32)
            for t in range(N_T):
                ut = u_tiles[t]
                uv = scr_pool.tile([128, JM], BF16, tag="uv")
                nc.vector.tensor_mul(uv[:], ut[:], v_rep[:])
                # m-reduction as a binary tree of 2x-mode TT adds
                uv3 = uv[:].rearrange("p (j m) -> p j m", m=32)
                t1 = t1_pool.tile([128, 1024], BF16)
                t13 = t1[:].rearrange("p (j m) -> p j m", m=16)
                nc.vector.tensor_add(t13, uv3[:, :, 0:16], uv3[:, :, 16:32])
                t2_ = t2_pool.tile([128, 512], BF16)
                t23 = t2_[:].rearrange("p (j m) -> p j m", m=8)
                nc.vector.tensor_add(t23, t13[:, :, 0:8], t13[:, :, 8:16])
                t3_ = t3_pool.tile([128, 256], BF16)
                t33 = t3_[:].rearrange("p (j m) -> p j m", m=4)
                nc.vector.tensor_add(t33, t23[:, :, 0:4], t23[:, :, 4:8])
                t4 = sm_pool.tile([128, 128], BF16, tag="t4")
                t43 = t4[:].rearrange("p (j m) -> p j m", m=2)
                nc.vector.tensor_add(t43, t33[:, :, 0:2], t33[:, :, 2:4])
                if r == 1:
                    a1 = a1_pool.tile([128, CH_J], F32)
                    a1_tiles.append(a1)
                    nc.vector.tensor_add(
                        a1[:].rearrange("p (j m) -> p j m", m=1),
                        t43[:, :, 0:1], t43[:, :, 1:2])
                    logits = a1
                else:
                    a2 = sm_pool.tile([128, CH_J], F32, tag="a2")
                    nc.vector.tensor_add(
                        a2[:].rearrange("p (j m) -> p j m", m=1),
                        t43[:, :, 0:1], t43[:, :, 1:2])
                    lg = sm_pool.tile([128, CH_J], F32, tag="lg")
                    nc.vector.tensor_add(lg[:], a2[:], a1_tiles[t][:])
                    logits = lg
                expt = sm_pool.tile([128, CH_J], F32, tag="expt")
                se = sm_pool.tile([128, 1], F32, tag="se")
                nc.scalar.activation(expt[:], logits[:], Act.Exp,
                                     scale=8.0, accum_out=se[:])
                se2 = sm_pool.tile([128, 1], F32, tag="se2")
                nc.vector.tensor_scalar_mul(se2[:], se[:], 1.0 / CH_J)
                rs = sm_pool.tile([128, 1], F32, tag="rs")
                nc.vector.reciprocal(rs[:], se2[:])
                ct = sm_pool.tile([128, CH_J], BF16, tag="ct")
                nc.vector.tensor_scalar_mul(ct[:], expt[:], rs[:])
                cexp = ce_pool.tile([128, JM], BF16, tag="cexp")
                nc.scalar.activation(
                    cexp[:].rearrange("p (j m) -> p j m", m=N_J),
                    ct[:].unsqueeze(-1).broadcast_to((128, CH_J, N_J)),
                    Act.Copy)
                cu = cu_pool.tile([128, JM], BF16, tag="cu")
                nc.vector.tensor_mul(cu[:], ut[:], cexp[:])
                for q in range(4):
                    nc.tensor.matmul(
                        s_ps[:, q * 512:(q + 1) * 512], s16[:],
                        cu[:, q * 512:(q + 1) * 512],
                        start=(t == 0), stop=(t == N_T - 1))
            s_sb2 = sv_pool.tile([16, JM], F32, tag="s_sb")
            nc.scalar.activation(s_sb2[:, :1024], s_ps[:, :1024], Act.Copy)
            nc.vector.tensor_copy(s_sb2[:, 1024:], s_ps[:, 1024:])
        v_sb = allreduce_squash(r, s_sb2, last=(r == 2))

    nc.sync.dma_start(v_d[:], v_sb[:])


def _host_inputs(inputs, w):
    """Build per-core input maps (host-side shard + block-diag pack)."""
    import ml_dtypes
    x = np.asarray(inputs, dtype=np.float32)
    w = np.asarray(w, dtype=np.float32)
    s16 = np.zeros((128, 16), dtype=np.float32)
    for i8 in range(8):
        for b in range(16):
            s16[i8 * 16 + b, b] = 1.0
    s16 = s16.astype(ml_dtypes.bfloat16)
    r8 = np.zeros((16, 128), dtype=np.float32)
    for i8 in range(8):
        for b in range(16):
            r8[b, i8 * 16 + b] = 1.0
    r8 = r8.astype(ml_dtypes.bfloat16)
    in_maps = []
    for k in range(N_CORES):
        i0 = k * I_LOC
        wk = w[i0:i0 + I_LOC].reshape(I_LOC * N_I, JM)
        wk = wk.astype(ml_dtypes.bfloat16)
        xk = x[:, i0:i0 + I_LOC, :]  # (B, 256, 16)
        xblk = np.zeros((N_T, 128, 128), dtype=np.float32)
        # xblk[t, i8*16+n, i8*16+b] = x[b, i0+8t+i8, n]
        xv = xk.transpose(1, 2, 0).reshape(N_T, 8, N_I, B)  # (t, i8, n, b)
        for i8 in range(8):
            xblk[:, i8 * 16:i8 * 16 + N_I, i8 * 16:i8 * 16 + B] = xv[:, i8]
        # xsum[t, i8*16+n, b] = x[b, i0+8t+i8, n]  (summed over i8 by matmul)
        xsum = xv.transpose(0, 1, 2, 3).reshape(N_T, 128, B).copy()
        in_maps.append({
            "xblk": xblk.astype(ml_dtypes.bfloat16),
            "xsum": xsum.astype(ml_dtypes.bfloat16),
            "w": wk, "s16": s16, "r8": r8})
    return in_maps


def kernel(inputs, w, _trace=False):
    key = "nc"
    if key not in _CACHE:
        _CACHE[key] = _build_program()
    nc = _CACHE[key]
    in_maps = _host_inputs(inputs, w)
    res = run_bass_kernel_spmd(nc, in_maps, list(range(N_CORES)),
                               trace=_trace)
    v = res.results[0]["v"].reshape(B, CH_J, N_J).astype(np.float32)
    if _trace:
        kernel._last = res
    return v


# revision 13
# speedup vs baseline: 1.3000x; 1.0190x over previous
"""DenseCaps routing kernel for 8x Trainium2 NeuronCores.

Shapes (hardcoded): inputs (16, 2048, 16) f32, w (2048, 16, 64, 32) f32.
Sharding: ch_i (2048) split 8 ways -> 256 i's per core. Each core computes
u[b, i_loc, j, m] via a block-diagonal stationary matmul streaming its w
shard once (bf16), keeps u resident in SBUF (bf16), runs the 3-iteration
dynamic routing locally, and AllReduces the tiny s[b, j, m] partial sums.
Output v (16, 64, 32) f32 is identical on all cores; core 0's is returned.
"""

import sys
from contextlib import ExitStack

import numpy as np

sys.path.insert(0, "/opt/trn_rl_repo")

import concourse.bass as bass
import concourse.bacc as bacc
import concourse.tile as tile
from concourse import mybir
from concourse.bass_utils import run_bass_kernel_spmd

F32 = mybir.dt.float32
BF16 = mybir.dt.bfloat16

B = 16
CH_I = 2048
N_I = 16
CH_J = 64
N_J = 32
JM = CH_J * N_J  # 2048
N_CORES = 8
I_LOC = CH_I // N_CORES  # 256
N_T = I_LOC // 8  # 32 production tiles, 8 i's each
EPS = 1e-7

_CACHE = {}


def _build_program(trace=False):
    nc = bacc.Bacc("TRN2", target_bir_lowering=False, debug=False,
                   num_devices=N_CORES)

    xblk_d = nc.dram_tensor("xblk", [N_T, 128, 128], BF16, kind="ExternalInput")
    xsum_d = nc.dram_tensor("xsum", [N_T, 128, 16], BF16, kind="ExternalInput")
    w_d = nc.dram_tensor("w", [I_LOC * N_I, JM], BF16, kind="ExternalInput")
    s16_d = nc.dram_tensor("s16", [128, 16], BF16, kind="ExternalInput")
    r8_d = nc.dram_tensor("r8", [16, 128], BF16, kind="ExternalInput")
    v_d = nc.dram_tensor("v", [B, JM], F32, kind="ExternalOutput")

    cc_in = [nc.dram_tensor(f"cc_in{r}", [B, JM], F32) for r in range(3)]
    cc_out = [nc.dram_tensor(f"cc_out{r}", [B, JM], F32, addr_space="Shared")
              for r in range(3)]

    with tile.TileContext(nc) as tc, ExitStack() as ctx:
        _kernel_body(ctx, tc, xblk_d, xsum_d, w_d, s16_d, r8_d, v_d,
                     cc_in, cc_out)
    nc.compile()
    return nc


def _kernel_body(ctx, tc, xblk_d, xsum_d, w_d, s16_d, r8_d, v_d,
                 cc_in, cc_out):
    nc = tc.nc
    Act = mybir.ActivationFunctionType
    Alu = mybir.AluOpType
    groups = [list(range(N_CORES))]

    const_pool = ctx.enter_context(tc.tile_pool(name="consts", bufs=1))
    s16 = const_pool.tile([128, 16], BF16)
    nc.sync.dma_start(s16[:], s16_d[:])
    r8 = const_pool.tile([16, 128], BF16)
    nc.sync.dma_start(r8[:], r8_d[:])
    eps_t = const_pool.tile([16, 1], F32)
    nc.vector.memset(eps_t[:], EPS)
    mln = const_pool.tile([128, 1], F32)
    nc.vector.memset(mln[:], -4.1588830833596715)  # -ln(CH_J)

    u_pool = ctx.enter_context(tc.tile_pool(name="u", bufs=N_T))
    u_tiles = []

    # small persistent buffers
    sv_pool = ctx.enter_context(tc.tile_pool(name="sv", bufs=1))
    a1_pool = ctx.enter_context(tc.tile_pool(name="a1", bufs=N_T))
    a1_tiles = []

    # ---------------- Phase A: produce u, accumulate s0 ----------------
    # s0 is accumulated straight from the w stream (stationary = per-tile
    # x summed over the 8 i-slots), so it does not depend on the u casts.
    with tc.tile_pool(name="wbuf", bufs=3) as w_pool, \
         tc.tile_pool(name="xbuf", bufs=3) as x_pool, \
         tc.tile_pool(name="xsbuf", bufs=3) as xs_pool, \
         tc.tile_pool(name="uprod", bufs=2, space="PSUM") as up_pool, \
         tc.tile_pool(name="s0ps", bufs=1, space="PSUM") as s0_pool:
        s0_ps = s0_pool.tile([16, JM], F32)
        for t in range(N_T):
            wt = w_pool.tile([128, JM], BF16)
            nc.sync.dma_start(wt[:], w_d[t * 128:(t + 1) * 128, :])
            xt = x_pool.tile([128, 128], BF16)
            nc.sync.dma_start(xt[:], xblk_d[t])
            xs = xs_pool.tile([128, 16], BF16)
            nc.sync.dma_start(xs[:], xsum_d[t])
            ut = u_pool.tile([128, JM], BF16)
            u_tiles.append(ut)
            for h in range(2):
                up = up_pool.tile([128, 1024], F32)
                for q in range(2):
                    o0 = h * 1024 + q * 512
                    nc.tensor.matmul(
                        up[:, q * 512:(q + 1) * 512], xt[:],
                        wt[:, o0:o0 + 512], start=True, stop=True)
                    nc.tensor.matmul(
                        s0_ps[:, o0:o0 + 512], xs[:], wt[:, o0:o0 + 512],
                        start=(t == 0), stop=(t == N_T - 1))
                dst = ut[:, h * 1024:(h + 1) * 1024]
                if h == 0:
                    nc.scalar.activation(dst, up[:], Act.Copy)
                else:
                    nc.vector.tensor_copy(dst, up[:])
        s_sb = sv_pool.tile([16, JM], F32, tag="s_sb")
        nc.scalar.activation(s_sb[:, :1024], s0_ps[:, :1024], Act.Copy)
        nc.vector.tensor_copy(s_sb[:, 1024:], s0_ps[:, 1024:])

    def allreduce_squash(r, s_sb, last):
        """AllReduce partial s, then squash -> v (bf16 for v_rep, or f32)."""
        nc.sync.dma_start(cc_in[r][:], s_sb[:])
        nc.gpsimd.collective_compute(
            "AllReduce", Alu.add, replica_groups=groups,
            ins=[cc_in[r][:]], outs=[cc_out[r][:]])
        sf = sv_pool.tile([16, JM], F32, tag="sf")
        nc.sync.dma_start(sf[:], cc_out[r][:])
        t2 = sv_pool.tile([16, JM], F32, tag="s_sb")
        nc.scalar.activation(t2[:], sf[:], Act.Square)
        sq = sv_pool.tile([16, CH_J], F32, tag="sq")
        nc.vector.tensor_reduce(
            sq[:], t2[:].rearrange("p (j m) -> p j m", m=N_J),
            axis=mybir.AxisListType.X, op=Alu.add)
        rt = sv_pool.tile([16, CH_J], F32, tag="rt")
        nc.scalar.activation(rt[:], sq[:], Act.Sqrt, bias=eps_t[:])
        onep = sv_pool.tile([16, CH_J], F32, tag="onep")
        nc.vector.tensor_scalar_add(onep[:], sq[:], 1.0)
        den = sv_pool.tile([16, CH_J], F32, tag="den")
        nc.vector.tensor_mul(den[:], rt[:], onep[:])
        rec = sv_pool.tile([16, CH_J], F32, tag="rec")
        nc.vector.reciprocal(rec[:], den[:])
        fs = sv_pool.tile([16, CH_J], F32, tag="fs")
        nc.vector.tensor_mul(fs[:], sq[:], rec[:])
        vv = sv_pool.tile([16, JM], F32 if last else BF16, tag="vv")
        nc.vector.tensor_mul(
            vv[:].rearrange("p (j m) -> p j m", m=N_J),
            sf[:].rearrange("p (j m) -> p j m", m=N_J),
            fs[:].unsqueeze(-1).broadcast_to((16, CH_J, N_J)))
        return vv

    v_sb = allreduce_squash(0, s_sb, last=False)

    # ---------------- Phases C: routing passes r=1,2 ----------------
    for r in (1, 2):
        with tc.tile_pool(name=f"vr{r}", bufs=1) as vr_pool, \
             tc.tile_pool(name=f"scr{r}", bufs=2) as scr_pool, \
             tc.tile_pool(name=f"ce{r}", bufs=2) as ce_pool, \
             tc.tile_pool(name=f"cu{r}", bufs=2) as cu_pool, \
             tc.tile_pool(name=f"t1_{r}", bufs=2) as t1_pool, \
             tc.tile_pool(name=f"t2_{r}", bufs=2) as t2_pool, \
             tc.tile_pool(name=f"t3_{r}", bufs=2) as t3_pool, \
             tc.tile_pool(name=f"sm{r}", bufs=3) as sm_pool, \
             tc.tile_pool(name=f"vrps{r}", bufs=2, space="PSUM") as vr_ps_pool, \
             tc.tile_pool(name=f"sps{r}", bufs=1, space="PSUM") as s_ps_pool:
            # v_rep (128, JM) bf16: broadcast v over the 8 i-slots
            v_rep = vr_pool.tile([128, JM], BF16)
            for h in range(2):
                vp = vr_ps_pool.tile([128, 1024], F32)
                for q in range(2):
                    nc.tensor.matmul(
                        vp[:, q * 512:(q + 1) * 512], r8[:],
                        v_sb[:, h * 1024 + q * 512:h * 1024 + (q + 1) * 512],
                        start=True, stop=True)
                if h == 0:
                    nc.scalar.activation(v_rep[:, h * 1024:(h + 1) * 1024],
                                         vp[:], Act.Copy)
                else:
                    nc.vector.tensor_copy(v_rep[:, h * 1024:(h + 1) * 1024],
                                          vp[:])
            s_ps = s_ps_pool.tile([16, JM], F32)
            for t in range(N_T):
                ut = u_tiles[t]
                uv = scr_pool.tile([128, JM], BF16, tag="uv")
                nc.vector.tensor_mul(uv[:], ut[:], v_rep[:])
                # m-reduction as a binary tree of 2x-mode TT adds
                uv3 = uv[:].rearrange("p (j m) -> p j m", m=32)
                t1 = t1_pool.tile([128, 1024], BF16)
                t13 = t1[:].rearrange("p (j m) -> p j m", m=16)
                nc.vector.tensor_add(t13, uv3[:, :, 0:16], uv3[:, :, 16:32])
                t2_ = t2_pool.tile([128, 512], BF16)
                t23 = t2_[:].rearrange("p (j m) -> p j m", m=8)
                nc.vector.tensor_add(t23, t13[:, :, 0:8], t13[:, :, 8:16])
                t3_ = t3_pool.tile([128, 256], BF16)
                t33 = t3_[:].rearrange("p (j m) -> p j m", m=4)
                nc.vector.tensor_add(t33, t23[:, :, 0:4], t23[:, :, 4:8])
                t4 = sm_pool.tile([128, 128], BF16, tag="t4")
                t43 = t4[:].rearrange("p (j m) -> p j m", m=2)
                nc.vector.tensor_add(t43, t33[:, :, 0:2], t33[:, :, 2:4])
                if r == 1:
                    a1 = a1_pool.tile([128, CH_J], F32)
                    a1_tiles.append(a1)
                    nc.vector.tensor_add(
                        a1[:].rearrange("p (j m) -> p j m", m=1),
                        t43[:, :, 0:1], t43[:, :, 1:2])
                    logits = a1
                else:
                    a2 = sm_pool.tile([128, CH_J], F32, tag="a2")
                    nc.vector.tensor_add(
                        a2[:].rearrange("p (j m) -> p j m", m=1),
                        t43[:, :, 0:1], t43[:, :, 1:2])
                    lg = sm_pool.tile([128, CH_J], F32, tag="lg")
                    nc.vector.tensor_add(lg[:], a2[:], a1_tiles[t][:])
                    logits = lg
                expt = sm_pool.tile([128, CH_J], BF16, tag="expt")
                se = sm_pool.tile([128, 1], F32, tag="se")
                nc.scalar.activation(expt[:], logits[:], Act.Exp,
                                     scale=8.0, bias=mln[:],
                                     accum_out=se[:])
                rs = sm_pool.tile([128, 1], F32, tag="rs")
                nc.vector.reciprocal(rs[:], se[:])
                cexp = ce_pool.tile([128, JM], BF16, tag="cexp")
                nc.scalar.activation(
                    cexp[:].rearrange("p (j m) -> p j m", m=N_J),
                    expt[:].unsqueeze(-1).broadcast_to((128, CH_J, N_J)),
                    Act.Copy, scale=rs[:])
                cu = cu_pool.tile([128, JM], BF16, tag="cu")
                nc.vector.tensor_mul(cu[:], ut[:], cexp[:])
                for q in range(4):
                    nc.tensor.matmul(
                        s_ps[:, q * 512:(q + 1) * 512], s16[:],
                        cu[:, q * 512:(q + 1) * 512],
                        start=(t == 0), stop=(t == N_T - 1))
            s_sb2 = sv_pool.tile([16, JM], F32, tag="s_sb")
            nc.scalar.activation(s_sb2[:, :1024], s_ps[:, :1024], Act.Copy)
            nc.vector.tensor_copy(s_sb2[:, 1024:], s_ps[:, 1024:])
        v_sb = allreduce_squash(r, s_sb2, last=(r == 2))

    nc.sync.dma_start(v_d[:], v_sb[:])


def _host_inputs(inputs, w):
    """Build per-core input maps (host-side shard + block-diag pack)."""
    import ml_dtypes
    x = np.asarray(inputs, dtype=np.float32)
    w = np.asarray(w, dtype=np.float32)
    # 64.0 (not 1.0): compensates the exp's folded-in 1/CH_J normalization
    s16 = np.zeros((128, 16), dtype=np.float32)
    for i8 in range(8):
        for b in range(16):
            s16[i8 * 16 + b, b] = 64.0
    s16 = s16.astype(ml_dtypes.bfloat16)
    r8 = np.zeros((16, 128), dtype=np.float32)
    for i8 in range(8):
        for b in range(16):
            r8[b, i8 * 16 + b] = 1.0
    r8 = r8.astype(ml_dtypes.bfloat16)
    in_maps = []
    for k in range(N_CORES):
        i0 = k * I_LOC
        wk = w[i0:i0 + I_LOC].reshape(I_LOC * N_I, JM)
        wk = wk.astype(ml_dtypes.bfloat16)
        xk = x[:, i0:i0 + I_LOC, :]  # (B, 256, 16)
        xblk = np.zeros((N_T, 128, 128), dtype=np.float32)
        # xblk[t, i8*16+n, i8*16+b] = x[b, i0+8t+i8, n]
        xv = xk.transpose(1, 2, 0).reshape(N_T, 8, N_I, B)  # (t, i8, n, b)
        for i8 in range(8):
            xblk[:, i8 * 16:i8 * 16 + N_I, i8 * 16:i8 * 16 + B] = xv[:, i8]
        # xsum[t, i8*16+n, b] = x[b, i0+8t+i8, n]  (summed over i8 by matmul)
        xsum = xv.transpose(0, 1, 2, 3).reshape(N_T, 128, B).copy()
        in_maps.append({
            "xblk": xblk.astype(ml_dtypes.bfloat16),
            "xsum": xsum.astype(ml_dtypes.bfloat16),
            "w": wk, "s16": s16, "r8": r8})
    return in_maps


def kernel(inputs, w, _trace=False):
    key = "nc"
    if key not in _CACHE:
        _CACHE[key] = _build_program()
    nc = _CACHE[key]
    in_maps = _host_inputs(inputs, w)
    res = run_bass_kernel_spmd(nc, in_maps, list(range(N_CORES)),
                               trace=_trace)
    v = res.results[0]["v"].reshape(B, CH_J, N_J).astype(np.float32)
    if _trace:
        kernel._last = res
    return v


# revision 20
# speedup vs baseline: 1.3360x; 1.0277x over previous
"""DenseCaps routing kernel for 8x Trainium2 NeuronCores.

Shapes (hardcoded): inputs (16, 2048, 16) f32, w (2048, 16, 64, 32) f32.
Sharding: ch_i (2048) split 8 ways -> 256 i's per core. Each core computes
u[b, i_loc, j, m] via a block-diagonal stationary matmul streaming its w
shard once (bf16), keeps u resident in SBUF (bf16), runs the 3-iteration
dynamic routing locally, and AllReduces the tiny s[b, j, m] partial sums.
Output v (16, 64, 32) f32 is identical on all cores; core 0's is returned.
"""

import sys
from contextlib import ExitStack

import numpy as np

sys.path.insert(0, "/opt/trn_rl_repo")

import concourse.bass as bass
import concourse.bacc as bacc
import concourse.tile as tile
from concourse import mybir
from concourse.bass_utils import run_bass_kernel_spmd

F32 = mybir.dt.float32
BF16 = mybir.dt.bfloat16

B = 16
CH_I = 2048
N_I = 16
CH_J = 64
N_J = 32
JM = CH_J * N_J  # 2048
N_CORES = 8
I_LOC = CH_I // N_CORES  # 256
N_T = I_LOC // 8  # 32 production tiles, 8 i's each
EPS = 1e-7

_CACHE = {}


def _build_program(trace=False):
    nc = bacc.Bacc("TRN2", target_bir_lowering=False, debug=False,
                   num_devices=N_CORES)

    xblk_d = nc.dram_tensor("xblk", [N_T, 128, 128], BF16, kind="ExternalInput")
    xsum_d = nc.dram_tensor("xsum", [N_T, 128, 16], BF16, kind="ExternalInput")
    w_d = nc.dram_tensor("w", [I_LOC * N_I, JM], BF16, kind="ExternalInput")
    s16_d = nc.dram_tensor("s16", [128, 16], BF16, kind="ExternalInput")
    r8_d = nc.dram_tensor("r8", [16, 128], BF16, kind="ExternalInput")
    v_d = nc.dram_tensor("v", [B, JM], F32, kind="ExternalOutput")

    cc_in = [nc.dram_tensor(f"cc_in{r}", [B, JM], F32) for r in range(4)]
    cc_out = [nc.dram_tensor(f"cc_out{r}", [B, JM], F32, addr_space="Shared")
              for r in range(4)]
    ccw_in = nc.dram_tensor("ccw_in", [16, 16], F32)
    ccw_out = nc.dram_tensor("ccw_out", [16, 16], F32, addr_space="Shared")

    with tile.TileContext(nc) as tc, ExitStack() as ctx:
        _kernel_body(ctx, tc, xblk_d, xsum_d, w_d, s16_d, r8_d, v_d,
                     cc_in, cc_out, ccw_in, ccw_out)
    nc.compile()
    return nc


def _kernel_body(ctx, tc, xblk_d, xsum_d, w_d, s16_d, r8_d, v_d,
                 cc_in, cc_out, ccw_in, ccw_out):
    nc = tc.nc
    Act = mybir.ActivationFunctionType
    Alu = mybir.AluOpType
    groups = [list(range(N_CORES))]

    # Tiny dummy AllReduce first: absorbs the ~20us first-collective warmup
    # while Phase A runs.
    nc.gpsimd.collective_compute(
        "AllReduce", Alu.add, replica_groups=groups,
        ins=[ccw_in[:]], outs=[ccw_out[:]])

    const_pool = ctx.enter_context(tc.tile_pool(name="consts", bufs=1))
    s16 = const_pool.tile([128, 16], BF16)
    nc.sync.dma_start(s16[:], s16_d[:])
    r8 = const_pool.tile([16, 128], BF16)
    nc.sync.dma_start(r8[:], r8_d[:])
    eps_t = const_pool.tile([16, 1], F32)
    nc.vector.memset(eps_t[:], EPS)
    mln = const_pool.tile([128, 1], F32)
    nc.vector.memset(mln[:], -4.1588830833596715)  # -ln(CH_J)

    u_pool = ctx.enter_context(tc.tile_pool(name="u", bufs=N_T))
    u_tiles = []

    # small persistent buffers
    sv_pool = ctx.enter_context(tc.tile_pool(name="sv", bufs=1))
    a1_pool = ctx.enter_context(tc.tile_pool(name="a1", bufs=N_T))
    a1_tiles = []

    # ---------------- Phase A: produce u, accumulate s0 ----------------
    # s0 is accumulated straight from the w stream (stationary = per-tile
    # x summed over the 8 i-slots), so it does not depend on the u casts.
    with tc.tile_pool(name="wbuf", bufs=3) as w_pool, \
         tc.tile_pool(name="xbuf", bufs=3) as x_pool, \
         tc.tile_pool(name="xsbuf", bufs=3) as xs_pool, \
         tc.tile_pool(name="uprod", bufs=2, space="PSUM") as up_pool, \
         tc.tile_pool(name="s0ps", bufs=1, space="PSUM") as s0_pool:
        HALF = N_T // 2
        s0_ps = s0_pool.tile([16, JM], F32, tag="s0")
        for t in range(N_T):
            if t == HALF:
                # first-half partial s0: ship its AllReduce while the
                # second half of Phase A computes
                s_sba = sv_pool.tile([16, JM], F32, tag="s_sba")
                nc.scalar.activation(s_sba[:, :1024], s0_ps[:, :1024],
                                     Act.Copy)
                nc.vector.tensor_copy(s_sba[:, 1024:], s0_ps[:, 1024:])
                nc.sync.dma_start(cc_in[0][:], s_sba[:])
                nc.gpsimd.collective_compute(
                    "AllReduce", Alu.add, replica_groups=groups,
                    ins=[cc_in[0][:]], outs=[cc_out[0][:]])
                s0_ps = s0_pool.tile([16, JM], F32, tag="s0")
            wt = w_pool.tile([128, JM], BF16)
            nc.sync.dma_start(wt[:], w_d[t * 128:(t + 1) * 128, :])
            xt = x_pool.tile([128, 128], BF16)
            nc.sync.dma_start(xt[:], xblk_d[t])
            xs = xs_pool.tile([128, 16], BF16)
            nc.sync.dma_start(xs[:], xsum_d[t])
            ut = u_pool.tile([128, JM], BF16)
            u_tiles.append(ut)
            for h in range(2):
                up = up_pool.tile([128, 1024], F32)
                for q in range(2):
                    o0 = h * 1024 + q * 512
                    nc.tensor.matmul(
                        up[:, q * 512:(q + 1) * 512], xt[:],
                        wt[:, o0:o0 + 512], start=True, stop=True)
                    nc.tensor.matmul(
                        s0_ps[:, o0:o0 + 512], xs[:], wt[:, o0:o0 + 512],
                        start=(t % HALF == 0), stop=(t % HALF == HALF - 1))
                dst = ut[:, h * 1024:(h + 1) * 1024]
                if h == 0:
                    nc.scalar.activation(dst, up[:], Act.Copy)
                else:
                    nc.vector.tensor_copy(dst, up[:])
        s_sb = sv_pool.tile([16, JM], F32, tag="s_sb")
        nc.scalar.activation(s_sb[:, :1024], s0_ps[:, :1024], Act.Copy)
        nc.vector.tensor_copy(s_sb[:, 1024:], s0_ps[:, 1024:])

    def allreduce_squash(r, s_sb, last):
        """AllReduce partial s, then squash -> v (bf16 for v_rep, or f32)."""
        ri = 3 if r == 0 else r  # slot 0 is taken by the first-half s0 AR
        nc.sync.dma_start(cc_in[ri][:], s_sb[:])
        nc.gpsimd.collective_compute(
            "AllReduce", Alu.add, replica_groups=groups,
            ins=[cc_in[ri][:]], outs=[cc_out[ri][:]])
        sf = sv_pool.tile([16, JM], F32, tag="sf")
        nc.sync.dma_start(sf[:], cc_out[ri][:])
        if r == 0:
            # fold in the first-half AllReduce result
            sfa = sv_pool.tile([16, JM], F32, tag="s_sba")
            nc.sync.dma_start(sfa[:], cc_out[0][:])
            nc.vector.tensor_add(sf[:], sf[:], sfa[:])
        t2 = sv_pool.tile([16, JM], F32, tag="s_sb")
        nc.scalar.activation(t2[:], sf[:], Act.Square)
        sq = sv_pool.tile([16, CH_J], F32, tag="sq")
        nc.vector.tensor_reduce(
            sq[:], t2[:].rearrange("p (j m) -> p j m", m=N_J),
            axis=mybir.AxisListType.X, op=Alu.add)
        rt = sv_pool.tile([16, CH_J], F32, tag="rt")
        nc.scalar.activation(rt[:], sq[:], Act.Sqrt, bias=eps_t[:])
        onep = sv_pool.tile([16, CH_J], F32, tag="onep")
        nc.vector.tensor_scalar_add(onep[:], sq[:], 1.0)
        den = sv_pool.tile([16, CH_J], F32, tag="den")
        nc.vector.tensor_mul(den[:], rt[:], onep[:])
        rec = sv_pool.tile([16, CH_J], F32, tag="rec")
        nc.vector.reciprocal(rec[:], den[:])
        fs = sv_pool.tile([16, CH_J], F32, tag="fs")
        nc.vector.tensor_mul(fs[:], sq[:], rec[:])
        vv = sv_pool.tile([16, JM], F32 if last else BF16, tag="vv")
        nc.vector.tensor_mul(
            vv[:].rearrange("p (j m) -> p j m", m=N_J),
            sf[:].rearrange("p (j m) -> p j m", m=N_J),
            fs[:].unsqueeze(-1).broadcast_to((16, CH_J, N_J)))
        return vv

    v_sb = allreduce_squash(0, s_sb, last=False)

    # ---------------- Phases C: routing passes r=1,2 ----------------
    for r in (1, 2):
        with tc.tile_pool(name=f"vr{r}", bufs=1) as vr_pool, \
             tc.tile_pool(name=f"scr{r}", bufs=2) as scr_pool, \
             tc.tile_pool(name=f"ce{r}", bufs=2) as ce_pool, \
             tc.tile_pool(name=f"cu{r}", bufs=2) as cu_pool, \
             tc.tile_pool(name=f"t1_{r}", bufs=2) as t1_pool, \
             tc.tile_pool(name=f"t2_{r}", bufs=2) as t2_pool, \
             tc.tile_pool(name=f"t3_{r}", bufs=2) as t3_pool, \
             tc.tile_pool(name=f"sm{r}", bufs=3) as sm_pool, \
             tc.tile_pool(name=f"vrps{r}", bufs=2, space="PSUM") as vr_ps_pool, \
             tc.tile_pool(name=f"sps{r}", bufs=1, space="PSUM") as s_ps_pool:
            # v_rep (128, JM) bf16: broadcast v over the 8 i-slots
            v_rep = vr_pool.tile([128, JM], BF16)
            for h in range(2):
                vp = vr_ps_pool.tile([128, 1024], F32)
                for q in range(2):
                    nc.tensor.matmul(
                        vp[:, q * 512:(q + 1) * 512], r8[:],
                        v_sb[:, h * 1024 + q * 512:h * 1024 + (q + 1) * 512],
                        start=True, stop=True)
                nc.scalar.activation(v_rep[:, h * 1024:(h + 1) * 1024],
                                     vp[:], Act.Copy)
            s_ps = s_ps_pool.tile([16, JM], F32)
            for t in range(N_T):
                ut = u_tiles[t]
                uv = scr_pool.tile([128, JM], BF16, tag="uv")
                nc.vector.tensor_mul(uv[:], ut[:], v_rep[:])
                # m-reduction as a binary tree of 2x-mode TT adds
                uv3 = uv[:].rearrange("p (j m) -> p j m", m=32)
                t1 = t1_pool.tile([128, 1024], BF16)
                t13 = t1[:].rearrange("p (j m) -> p j m", m=16)
                nc.vector.tensor_add(t13, uv3[:, :, 0:16], uv3[:, :, 16:32])
                t2_ = t2_pool.tile([128, 512], BF16)
                t23 = t2_[:].rearrange("p (j m) -> p j m", m=8)
                nc.vector.tensor_add(t23, t13[:, :, 0:8], t13[:, :, 8:16])
                t3_ = t3_pool.tile([128, 256], BF16)
                t33 = t3_[:].rearrange("p (j m) -> p j m", m=4)
                nc.vector.tensor_add(t33, t23[:, :, 0:4], t23[:, :, 4:8])
                t4 = sm_pool.tile([128, 128], BF16, tag="t4")
                t43 = t4[:].rearrange("p (j m) -> p j m", m=2)
                nc.vector.tensor_add(t43, t33[:, :, 0:2], t33[:, :, 2:4])
                if r == 1:
                    a1 = a1_pool.tile([128, CH_J], F32)
                    a1_tiles.append(a1)
                    nc.vector.tensor_add(
                        a1[:].rearrange("p (j m) -> p j m", m=1),
                        t43[:, :, 0:1], t43[:, :, 1:2])
                    logits = a1
                else:
                    a2 = sm_pool.tile([128, CH_J], F32, tag="a2")
                    nc.vector.tensor_add(
                        a2[:].rearrange("p (j m) -> p j m", m=1),
                        t43[:, :, 0:1], t43[:, :, 1:2])
                    lg = sm_pool.tile([128, CH_J], F32, tag="lg")
                    nc.vector.tensor_add(lg[:], a2[:], a1_tiles[t][:])
                    logits = lg
                expt = sm_pool.tile([128, CH_J], BF16, tag="expt")
                se = sm_pool.tile([128, 1], F32, tag="se")
                nc.scalar.activation(expt[:], logits[:], Act.Exp,
                                     scale=8.0, bias=mln[:],
                                     accum_out=se[:])
                rs = sm_pool.tile([128, 1], F32, tag="rs")
                nc.vector.reciprocal(rs[:], se[:])
                cexp = ce_pool.tile([128, JM], BF16, tag="cexp")
                nc.scalar.activation(
                    cexp[:].rearrange("p (j m) -> p j m", m=N_J),
                    expt[:].unsqueeze(-1).broadcast_to((128, CH_J, N_J)),
                    Act.Copy, scale=rs[:])
                cu = cu_pool.tile([128, JM], BF16, tag="cu")
                nc.vector.tensor_mul(cu[:], ut[:], cexp[:])
                for q in range(4):
                    nc.tensor.matmul(
                        s_ps[:, q * 512:(q + 1) * 512], s16[:],
                        cu[:, q * 512:(q + 1) * 512],
                        start=(t == 0), stop=(t == N_T - 1))
            s_sb2 = sv_pool.tile([16, JM], F32, tag="s_sb")
            nc.scalar.activation(s_sb2[:], s_ps[:], Act.Copy)
        v_sb = allreduce_squash(r, s_sb2, last=(r == 2))

    nc.sync.dma_start(v_d[:], v_sb[:])


def _host_inputs(inputs, w):
    """Build per-core input maps (host-side shard + block-diag pack)."""
    import ml_dtypes
    x = np.asarray(inputs, dtype=np.float32)
    w = np.asarray(w, dtype=np.float32)
    # 64.0 (not 1.0): compensates the exp's folded-in 1/CH_J normalization
    s16 = np.zeros((128, 16), dtype=np.float32)
    for i8 in range(8):
        for b in range(16):
            s16[i8 * 16 + b, b] = 64.0
    s16 = s16.astype(ml_dtypes.bfloat16)
    r8 = np.zeros((16, 128), dtype=np.float32)
    for i8 in range(8):
        for b in range(16):
            r8[b, i8 * 16 + b] = 1.0
    r8 = r8.astype(ml_dtypes.bfloat16)
    in_maps = []
    for k in range(N_CORES):
        i0 = k * I_LOC
        wk = w[i0:i0 + I_LOC].reshape(I_LOC * N_I, JM)
        wk = wk.astype(ml_dtypes.bfloat16)
        xk = x[:, i0:i0 + I_LOC, :]  # (B, 256, 16)
        xblk = np.zeros((N_T, 128, 128), dtype=np.float32)
        # xblk[t, i8*16+n, i8*16+b] = x[b, i0+8t+i8, n]
        xv = xk.transpose(1, 2, 0).reshape(N_T, 8, N_I, B)  # (t, i8, n, b)
        for i8 in range(8):
            xblk[:, i8 * 16:i8 * 16 + N_I, i8 * 16:i8 * 16 + B] = xv[:, i8]
        # xsum[t, i8*16+n, b] = x[b, i0+8t+i8, n]  (summed over i8 by matmul)
        xsum = xv.transpose(0, 1, 2, 3).reshape(N_T, 128, B).copy()
        in_maps.append({
            "xblk": xblk.astype(ml_dtypes.bfloat16),
            "xsum": xsum.astype(ml_dtypes.bfloat16),
            "w": wk, "s16": s16, "r8": r8})
    return in_maps


def kernel(inputs, w, _trace=False):
    key = "nc"
    if key not in _CACHE:
        _CACHE[key] = _build_program()
    nc = _CACHE[key]
    in_maps = _host_inputs(inputs, w)
    res = run_bass_kernel_spmd(nc, in_maps, list(range(N_CORES)),
                               trace=_trace)
    v = res.results[0]["v"].reshape(B, CH_J, N_J).astype(np.float32)
    if _trace:
        kernel._last = res
    return v


# revision 21
# speedup vs baseline: 1.3398x; 1.0028x over previous
"""DenseCaps routing kernel for 8x Trainium2 NeuronCores.

Shapes (hardcoded): inputs (16, 2048, 16) f32, w (2048, 16, 64, 32) f32.
Sharding: ch_i (2048) split 8 ways -> 256 i's per core. Each core computes
u[b, i_loc, j, m] via a block-diagonal stationary matmul streaming its w
shard once (bf16), keeps u resident in SBUF (bf16), runs the 3-iteration
dynamic routing locally, and AllReduces the tiny s[b, j, m] partial sums.
Output v (16, 64, 32) f32 is identical on all cores; core 0's is returned.
"""

import sys
from contextlib import ExitStack

import numpy as np

sys.path.insert(0, "/opt/trn_rl_repo")

import concourse.bass as bass
import concourse.bacc as bacc
import concourse.tile as tile
from concourse import mybir
from concourse.bass_utils import run_bass_kernel_spmd

F32 = mybir.dt.float32
BF16 = mybir.dt.bfloat16

B = 16
CH_I = 2048
N_I = 16
CH_J = 64
N_J = 32
JM = CH_J * N_J  # 2048
N_CORES = 8
I_LOC = CH_I // N_CORES  # 256
N_T = I_LOC // 8  # 32 production tiles, 8 i's each
EPS = 1e-7

_CACHE = {}


def _build_program(trace=False):
    nc = bacc.Bacc("TRN2", target_bir_lowering=False, debug=False,
                   num_devices=N_CORES)

    xblk_d = nc.dram_tensor("xblk", [N_T, 128, 128], BF16, kind="ExternalInput")
    xsum_d = nc.dram_tensor("xsum", [N_T, 128, 16], BF16, kind="ExternalInput")
    w_d = nc.dram_tensor("w", [I_LOC * N_I, JM], BF16, kind="ExternalInput")
    s16_d = nc.dram_tensor("s16", [128, 16], BF16, kind="ExternalInput")
    r8_d = nc.dram_tensor("r8", [16, 128], BF16, kind="ExternalInput")
    v_d = nc.dram_tensor("v", [B, JM], F32, kind="ExternalOutput")

    cc_in = [nc.dram_tensor(f"cc_in{r}", [B, JM], F32) for r in range(4)]
    cc_out = [nc.dram_tensor(f"cc_out{r}", [B, JM], F32, addr_space="Shared")
              for r in range(4)]
    ccw_in = nc.dram_tensor("ccw_in", [16, 16], F32)
    ccw_out = nc.dram_tensor("ccw_out", [16, 16], F32, addr_space="Shared")

    with tile.TileContext(nc) as tc, ExitStack() as ctx:
        _kernel_body(ctx, tc, xblk_d, xsum_d, w_d, s16_d, r8_d, v_d,
                     cc_in, cc_out, ccw_in, ccw_out)
    nc.compile()
    return nc


def _kernel_body(ctx, tc, xblk_d, xsum_d, w_d, s16_d, r8_d, v_d,
                 cc_in, cc_out, ccw_in, ccw_out):
    nc = tc.nc
    Act = mybir.ActivationFunctionType
    Alu = mybir.AluOpType
    groups = [list(range(N_CORES))]

    WARMUP_AR = False
    if WARMUP_AR:
        nc.gpsimd.collective_compute(
            "AllReduce", Alu.add, replica_groups=groups,
            ins=[ccw_in[:]], outs=[ccw_out[:]])

    const_pool = ctx.enter_context(tc.tile_pool(name="consts", bufs=1))
    s16 = const_pool.tile([128, 16], BF16)
    nc.sync.dma_start(s16[:], s16_d[:])
    r8 = const_pool.tile([16, 128], BF16)
    nc.sync.dma_start(r8[:], r8_d[:])
    eps_t = const_pool.tile([16, 1], F32)
    nc.vector.memset(eps_t[:], EPS)
    mln = const_pool.tile([128, 1], F32)
    nc.vector.memset(mln[:], -4.1588830833596715)  # -ln(CH_J)

    u_pool = ctx.enter_context(tc.tile_pool(name="u", bufs=N_T))
    u_tiles = []

    # small persistent buffers
    sv_pool = ctx.enter_context(tc.tile_pool(name="sv", bufs=1))
    a1_pool = ctx.enter_context(tc.tile_pool(name="a1", bufs=N_T))
    a1_tiles = []

    # ---------------- Phase A: produce u, accumulate s0 ----------------
    # s0 is accumulated straight from the w stream (stationary = per-tile
    # x summed over the 8 i-slots), so it does not depend on the u casts.
    with tc.tile_pool(name="wbuf", bufs=3) as w_pool, \
         tc.tile_pool(name="xbuf", bufs=3) as x_pool, \
         tc.tile_pool(name="xsbuf", bufs=3) as xs_pool, \
         tc.tile_pool(name="uprod", bufs=2, space="PSUM") as up_pool, \
         tc.tile_pool(name="s0ps", bufs=1, space="PSUM") as s0_pool:
        HALF = N_T // 2
        s0_ps = s0_pool.tile([16, JM], F32, tag="s0")
        for t in range(N_T):
            if t == HALF:
                # first-half partial s0: ship its AllReduce while the
                # second half of Phase A computes
                s_sba = sv_pool.tile([16, JM], F32, tag="s_sba")
                nc.scalar.activation(s_sba[:, :1024], s0_ps[:, :1024],
                                     Act.Copy)
                nc.vector.tensor_copy(s_sba[:, 1024:], s0_ps[:, 1024:])
                nc.sync.dma_start(cc_in[0][:], s_sba[:])
                nc.gpsimd.collective_compute(
                    "AllReduce", Alu.add, replica_groups=groups,
                    ins=[cc_in[0][:]], outs=[cc_out[0][:]])
                s0_ps = s0_pool.tile([16, JM], F32, tag="s0")
            wt = w_pool.tile([128, JM], BF16)
            nc.sync.dma_start(wt[:], w_d[t * 128:(t + 1) * 128, :])
            xt = x_pool.tile([128, 128], BF16)
            nc.sync.dma_start(xt[:], xblk_d[t])
            xs = xs_pool.tile([128, 16], BF16)
            nc.sync.dma_start(xs[:], xsum_d[t])
            ut = u_pool.tile([128, JM], BF16)
            u_tiles.append(ut)
            for h in range(2):
                up = up_pool.tile([128, 1024], F32)
                for q in range(2):
                    o0 = h * 1024 + q * 512
                    nc.tensor.matmul(
                        up[:, q * 512:(q + 1) * 512], xt[:],
                        wt[:, o0:o0 + 512], start=True, stop=True)
                    nc.tensor.matmul(
                        s0_ps[:, o0:o0 + 512], xs[:], wt[:, o0:o0 + 512],
                        start=(t % HALF == 0), stop=(t % HALF == HALF - 1))
                dst = ut[:, h * 1024:(h + 1) * 1024]
                if h == 0:
                    nc.scalar.activation(dst, up[:], Act.Copy)
                else:
                    nc.vector.tensor_copy(dst, up[:])
        s_sb = sv_pool.tile([16, JM], F32, tag="s_sb")
        nc.scalar.activation(s_sb[:, :1024], s0_ps[:, :1024], Act.Copy)
        nc.vector.tensor_copy(s_sb[:, 1024:], s0_ps[:, 1024:])

    def allreduce_squash(r, s_sb, last):
        """AllReduce partial s, then squash -> v (bf16 for v_rep, or f32)."""
        ri = 3 if r == 0 else r  # slot 0 is taken by the first-half s0 AR
        nc.sync.dma_start(cc_in[ri][:], s_sb[:])
        nc.gpsimd.collective_compute(
            "AllReduce", Alu.add, replica_groups=groups,
            ins=[cc_in[ri][:]], outs=[cc_out[ri][:]])
        sf = sv_pool.tile([16, JM], F32, tag="sf")
        nc.sync.dma_start(sf[:], cc_out[ri][:])
        if r == 0:
            # fold in the first-half AllReduce result
            sfa = sv_pool.tile([16, JM], F32, tag="s_sba")
            nc.sync.dma_start(sfa[:], cc_out[0][:])
            nc.vector.tensor_add(sf[:], sf[:], sfa[:])
        t2 = sv_pool.tile([16, JM], F32, tag="s_sb")
        nc.scalar.activation(t2[:], sf[:], Act.Square)
        sq = sv_pool.tile([16, CH_J], F32, tag="sq")
        nc.vector.tensor_reduce(
            sq[:], t2[:].rearrange("p (j m) -> p j m", m=N_J),
            axis=mybir.AxisListType.X, op=Alu.add)
        rt = sv_pool.tile([16, CH_J], F32, tag="rt")
        nc.scalar.activation(rt[:], sq[:], Act.Sqrt, bias=eps_t[:])
        onep = sv_pool.tile([16, CH_J], F32, tag="onep")
        nc.vector.tensor_scalar_add(onep[:], sq[:], 1.0)
        den = sv_pool.tile([16, CH_J], F32, tag="den")
        nc.vector.tensor_mul(den[:], rt[:], onep[:])
        rec = sv_pool.tile([16, CH_J], F32, tag="rec")
        nc.vector.reciprocal(rec[:], den[:])
        fs = sv_pool.tile([16, CH_J], F32, tag="fs")
        nc.vector.tensor_mul(fs[:], sq[:], rec[:])
        vv = sv_pool.tile([16, JM], F32 if last else BF16, tag="vv")
        nc.vector.tensor_mul(
            vv[:].rearrange("p (j m) -> p j m", m=N_J),
            sf[:].rearrange("p (j m) -> p j m", m=N_J),
            fs[:].unsqueeze(-1).broadcast_to((16, CH_J, N_J)))
        return vv

    v_sb = allreduce_squash(0, s_sb, last=False)

    # ---------------- Phases C: routing passes r=1,2 ----------------
    for r in (1, 2):
        with tc.tile_pool(name=f"vr{r}", bufs=1) as vr_pool, \
             tc.tile_pool(name=f"scr{r}", bufs=2) as scr_pool, \
             tc.tile_pool(name=f"ce{r}", bufs=2) as ce_pool, \
             tc.tile_pool(name=f"cu{r}", bufs=2) as cu_pool, \
             tc.tile_pool(name=f"t1_{r}", bufs=2) as t1_pool, \
             tc.tile_pool(name=f"t2_{r}", bufs=2) as t2_pool, \
             tc.tile_pool(name=f"t3_{r}", bufs=2) as t3_pool, \
             tc.tile_pool(name=f"sm{r}", bufs=3) as sm_pool, \
             tc.tile_pool(name=f"vrps{r}", bufs=2, space="PSUM") as vr_ps_pool, \
             tc.tile_pool(name=f"sps{r}", bufs=1, space="PSUM") as s_ps_pool:
            # v_rep (128, JM) bf16: broadcast v over the 8 i-slots
            v_rep = vr_pool.tile([128, JM], BF16)
            for h in range(2):
                vp = vr_ps_pool.tile([128, 1024], F32)
                for q in range(2):
                    nc.tensor.matmul(
                        vp[:, q * 512:(q + 1) * 512], r8[:],
                        v_sb[:, h * 1024 + q * 512:h * 1024 + (q + 1) * 512],
                        start=True, stop=True)
                nc.scalar.activation(v_rep[:, h * 1024:(h + 1) * 1024],
                                     vp[:], Act.Copy)
            s_ps = s_ps_pool.tile([16, JM], F32)
            for t in range(N_T):
                ut = u_tiles[t]
                uv = scr_pool.tile([128, JM], BF16, tag="uv")
                nc.vector.tensor_mul(uv[:], ut[:], v_rep[:])
                # m-reduction as a binary tree of 2x-mode TT adds
                uv3 = uv[:].rearrange("p (j m) -> p j m", m=32)
                t1 = t1_pool.tile([128, 1024], BF16)
                t13 = t1[:].rearrange("p (j m) -> p j m", m=16)
                nc.vector.tensor_add(t13, uv3[:, :, 0:16], uv3[:, :, 16:32])
                t2_ = t2_pool.tile([128, 512], BF16)
                t23 = t2_[:].rearrange("p (j m) -> p j m", m=8)
                nc.vector.tensor_add(t23, t13[:, :, 0:8], t13[:, :, 8:16])
                t3_ = t3_pool.tile([128, 256], BF16)
                t33 = t3_[:].rearrange("p (j m) -> p j m", m=4)
                nc.vector.tensor_add(t33, t23[:, :, 0:4], t23[:, :, 4:8])
                t4 = sm_pool.tile([128, 128], BF16, tag="t4")
                t43 = t4[:].rearrange("p (j m) -> p j m", m=2)
                nc.vector.tensor_add(t43, t33[:, :, 0:2], t33[:, :, 2:4])
                if r == 1:
                    a1 = a1_pool.tile([128, CH_J], F32)
                    a1_tiles.append(a1)
                    nc.vector.tensor_add(
                        a1[:].rearrange("p (j m) -> p j m", m=1),
                        t43[:, :, 0:1], t43[:, :, 1:2])
                    logits = a1
                else:
                    a2 = sm_pool.tile([128, CH_J], F32, tag="a2")
                    nc.vector.tensor_add(
                        a2[:].rearrange("p (j m) -> p j m", m=1),
                        t43[:, :, 0:1], t43[:, :, 1:2])
                    lg = sm_pool.tile([128, CH_J], F32, tag="lg")
                    nc.vector.tensor_add(lg[:], a2[:], a1_tiles[t][:])
                    logits = lg
                expt = sm_pool.tile([128, CH_J], BF16, tag="expt")
                se = sm_pool.tile([128, 1], F32, tag="se")
                nc.scalar.activation(expt[:], logits[:], Act.Exp,
                                     scale=8.0, bias=mln[:],
                                     accum_out=se[:])
                rs = sm_pool.tile([128, 1], F32, tag="rs")
                nc.vector.reciprocal(rs[:], se[:])
                cexp = ce_pool.tile([128, JM], BF16, tag="cexp")
                nc.scalar.activation(
                    cexp[:].rearrange("p (j m) -> p j m", m=N_J),
                    expt[:].unsqueeze(-1).broadcast_to((128, CH_J, N_J)),
                    Act.Copy, scale=rs[:])
                cu = cu_pool.tile([128, JM], BF16, tag="cu")
                nc.vector.tensor_mul(cu[:], ut[:], cexp[:])
                for q in range(4):
                    nc.tensor.matmul(
                        s_ps[:, q * 512:(q + 1) * 512], s16[:],
                        cu[:, q * 512:(q + 1) * 512],
                        start=(t == 0), stop=(t == N_T - 1))
            s_sb2 = sv_pool.tile([16, JM], F32, tag="s_sb")
            nc.scalar.activation(s_sb2[:], s_ps[:], Act.Copy)
        v_sb = allreduce_squash(r, s_sb2, last=(r == 2))

    nc.sync.dma_start(v_d[:], v_sb[:])


def _host_inputs(inputs, w):
    """Build per-core input maps (host-side shard + block-diag pack)."""
    import ml_dtypes
    x = np.asarray(inputs, dtype=np.float32)
    w = np.asarray(w, dtype=np.float32)
    # 64.0 (not 1.0): compensates the exp's folded-in 1/CH_J normalization
    s16 = np.zeros((128, 16), dtype=np.float32)
    for i8 in range(8):
        for b in range(16):
            s16[i8 * 16 + b, b] = 64.0
    s16 = s16.astype(ml_dtypes.bfloat16)
    r8 = np.zeros((16, 128), dtype=np.float32)
    for i8 in range(8):
        for b in range(16):
            r8[b, i8 * 16 + b] = 1.0
    r8 = r8.astype(ml_dtypes.bfloat16)
    in_maps = []
    for k in range(N_CORES):
        i0 = k * I_LOC
        wk = w[i0:i0 + I_LOC].reshape(I_LOC * N_I, JM)
        wk = wk.astype(ml_dtypes.bfloat16)
        xk = x[:, i0:i0 + I_LOC, :]  # (B, 256, 16)
        xblk = np.zeros((N_T, 128, 128), dtype=np.float32)
        # xblk[t, i8*16+n, i8*16+b] = x[b, i0+8t+i8, n]
        xv = xk.transpose(1, 2, 0).reshape(N_T, 8, N_I, B)  # (t, i8, n, b)
        for i8 in range(8):
            xblk[:, i8 * 16:i8 * 16 + N_I, i8 * 16:i8 * 16 + B] = xv[:, i8]
        # xsum[t, i8*16+n, b] = x[b, i0+8t+i8, n]  (summed over i8 by matmul)
        xsum = xv.transpose(0, 1, 2, 3).reshape(N_T, 128, B).copy()
        in_maps.append({
            "xblk": xblk.astype(ml_dtypes.bfloat16),
            "xsum": xsum.astype(ml_dtypes.bfloat16),
            "w": wk, "s16": s16, "r8": r8})
    return in_maps


def kernel(inputs, w, _trace=False):
    key = "nc"
    if key not in _CACHE:
        _CACHE[key] = _build_program()
    nc = _CACHE[key]
    in_maps = _host_inputs(inputs, w)
    res = run_bass_kernel_spmd(nc, in_maps, list(range(N_CORES)),
                               trace=_trace)
    v = res.results[0]["v"].reshape(B, CH_J, N_J).astype(np.float32)
    if _trace:
        kernel._last = res
    return v


# revision 27
# speedup vs baseline: 1.3645x; 1.0185x over previous
"""DenseCaps routing kernel for 8x Trainium2 NeuronCores.

Shapes (hardcoded): inputs (16, 2048, 16) f32, w (2048, 16, 64, 32) f32.
Sharding: ch_i (2048) split 8 ways -> 256 i's per core. Each core computes
u[b, i_loc, j, m] via a block-diagonal stationary matmul streaming its w
shard once (bf16), keeps u resident in SBUF (bf16), runs the 3-iteration
dynamic routing locally, and AllReduces the tiny s[b, j, m] partial sums.
Output v (16, 64, 32) f32 is identical on all cores; core 0's is returned.
"""

import sys
from contextlib import ExitStack

import numpy as np

sys.path.insert(0, "/opt/trn_rl_repo")

import concourse.bass as bass
import concourse.bacc as bacc
import concourse.tile as tile
from concourse import mybir
from concourse.bass_utils import run_bass_kernel_spmd

F32 = mybir.dt.float32
BF16 = mybir.dt.bfloat16

B = 16
CH_I = 2048
N_I = 16
CH_J = 64
N_J = 32
JM = CH_J * N_J  # 2048
N_CORES = 8
I_LOC = CH_I // N_CORES  # 256
N_T = I_LOC // 8  # 32 production tiles, 8 i's each
EPS = 1e-7

_CACHE = {}


def _build_program(trace=False):
    nc = bacc.Bacc("TRN2", target_bir_lowering=False, debug=False,
                   num_devices=N_CORES)

    xblk_d = nc.dram_tensor("xblk", [N_T, 128, 128], BF16, kind="ExternalInput")
    xsum_d = nc.dram_tensor("xsum", [N_T, 128, 16], BF16, kind="ExternalInput")
    w_d = nc.dram_tensor("w", [I_LOC * N_I, JM], BF16, kind="ExternalInput")
    s16_d = nc.dram_tensor("s16", [128, 16], BF16, kind="ExternalInput")
    r8_d = nc.dram_tensor("r8", [16, 128], BF16, kind="ExternalInput")
    v_d = nc.dram_tensor("v", [B, JM], F32, kind="ExternalOutput")

    cc_in = [nc.dram_tensor(f"cc_in{r}", [B, JM], F32) for r in range(5)]
    cc_out = [nc.dram_tensor(f"cc_out{r}", [B, JM], F32, addr_space="Shared")
              for r in range(5)]
    ccw_in = nc.dram_tensor("ccw_in", [16, 16], F32)
    ccw_out = nc.dram_tensor("ccw_out", [16, 16], F32, addr_space="Shared")

    with tile.TileContext(nc) as tc, ExitStack() as ctx:
        _kernel_body(ctx, tc, xblk_d, xsum_d, w_d, s16_d, r8_d, v_d,
                     cc_in, cc_out, ccw_in, ccw_out)
    nc.compile()
    return nc


def _kernel_body(ctx, tc, xblk_d, xsum_d, w_d, s16_d, r8_d, v_d,
                 cc_in, cc_out, ccw_in, ccw_out):
    nc = tc.nc
    Act = mybir.ActivationFunctionType
    Alu = mybir.AluOpType
    groups = [list(range(N_CORES))]

    WARMUP_AR = False
    if WARMUP_AR:
        nc.gpsimd.collective_compute(
            "AllReduce", Alu.add, replica_groups=groups,
            ins=[ccw_in[:]], outs=[ccw_out[:]])

    const_pool = ctx.enter_context(tc.tile_pool(name="consts", bufs=1))
    s16 = const_pool.tile([128, 16], BF16)
    nc.sync.dma_start(s16[:], s16_d[:])
    r8 = const_pool.tile([16, 128], BF16)
    nc.sync.dma_start(r8[:], r8_d[:])
    eps_t = const_pool.tile([16, 1], F32)
    nc.vector.memset(eps_t[:], EPS)
    mln = const_pool.tile([128, 1], F32)
    nc.vector.memset(mln[:], -4.1588830833596715)  # -ln(CH_J)

    u_pool = ctx.enter_context(tc.tile_pool(name="u", bufs=N_T))
    u_tiles = []

    # small persistent buffers
    sv_pool = ctx.enter_context(tc.tile_pool(name="sv", bufs=1))
    a1_pool = ctx.enter_context(tc.tile_pool(name="a1", bufs=N_T))
    a1_tiles = []

    # ---------------- Phase A: produce u, accumulate s0 ----------------
    # s0 is accumulated straight from the w stream (stationary = per-tile
    # x summed over the 8 i-slots), so it does not depend on the u casts.
    with tc.tile_pool(name="wbuf", bufs=3) as w_pool, \
         tc.tile_pool(name="xbuf", bufs=3) as x_pool, \
         tc.tile_pool(name="xsbuf", bufs=3) as xs_pool, \
         tc.tile_pool(name="uprod", bufs=2, space="PSUM") as up_pool, \
         tc.tile_pool(name="s0ps", bufs=1, space="PSUM") as s0_pool:
        # s0 in 3 chunks: the first AllReduce (cold: CC startup) triggers
        # ~40us in, so all three hide under Phase A compute.
        seg_end = {12: 0}  # boundary tile -> cc slot
        seg_start = 0
        s0_ps = s0_pool.tile([16, JM], F32, tag="s0")
        for t in range(N_T):
            if t in seg_end:
                slot = seg_end[t]
                s_sba = sv_pool.tile([16, JM], F32, tag=f"s_sb{slot}")
                nc.scalar.activation(s_sba[:, :1024], s0_ps[:, :1024],
                                     Act.Copy)
                nc.vector.tensor_copy(s_sba[:, 1024:], s0_ps[:, 1024:])
                nc.sync.dma_start(cc_in[slot][:], s_sba[:])
                nc.gpsimd.collective_compute(
                    "AllReduce", Alu.add, replica_groups=groups,
                    ins=[cc_in[slot][:]], outs=[cc_out[slot][:]])
                s0_ps = s0_pool.tile([16, JM], F32, tag="s0")
                seg_start = t
            wt = w_pool.tile([128, JM], BF16)
            nc.sync.dma_start(wt[:], w_d[t * 128:(t + 1) * 128, :])
            xt = x_pool.tile([128, 128], BF16)
            nc.sync.dma_start(xt[:], xblk_d[t])
            xs = xs_pool.tile([128, 16], BF16)
            nc.sync.dma_start(xs[:], xsum_d[t])
            ut = u_pool.tile([128, JM], BF16)
            u_tiles.append(ut)
            for h in range(2):
                up = up_pool.tile([128, 1024], F32)
                for q in range(2):
                    o0 = h * 1024 + q * 512
                    nc.tensor.matmul(
                        up[:, q * 512:(q + 1) * 512], xt[:],
                        wt[:, o0:o0 + 512], start=True, stop=True)
                    nc.tensor.matmul(
                        s0_ps[:, o0:o0 + 512], xs[:], wt[:, o0:o0 + 512],
                        start=(t == seg_start),
                        stop=(t + 1 in seg_end or t == N_T - 1))
                dst = ut[:, h * 1024:(h + 1) * 1024]
                if h == 0:
                    nc.scalar.activation(dst, up[:], Act.Copy)
                else:
                    nc.vector.tensor_copy(dst, up[:])
        s_sb = sv_pool.tile([16, JM], F32, tag="s_sb")
        nc.scalar.activation(s_sb[:, :1024], s0_ps[:, :1024], Act.Copy)
        nc.vector.tensor_copy(s_sb[:, 1024:], s0_ps[:, 1024:])

    def allreduce_squash(r, s_sb, last):
        """AllReduce partial s, then squash -> v (bf16 for v_rep, or f32)."""
        ri = 4 if r == 0 else r  # slots 0/3 hold the early s0 chunk ARs
        nc.sync.dma_start(cc_in[ri][:], s_sb[:])
        nc.gpsimd.collective_compute(
            "AllReduce", Alu.add, replica_groups=groups,
            ins=[cc_in[ri][:]], outs=[cc_out[ri][:]])
        sf = sv_pool.tile([16, JM], F32, tag="sf")
        nc.sync.dma_start(sf[:], cc_out[ri][:])
        if r == 0:
            # fold in the early s0 chunk AllReduce result
            for slot in (0,):
                sfa = sv_pool.tile([16, JM], F32, tag=f"s_sb{slot}")
                nc.sync.dma_start(sfa[:], cc_out[slot][:])
                nc.vector.tensor_add(sf[:], sf[:], sfa[:])
        t2 = sv_pool.tile([16, JM], F32, tag="s_sb")
        nc.scalar.activation(t2[:], sf[:], Act.Square)
        sq = sv_pool.tile([16, CH_J], F32, tag="sq")
        nc.vector.tensor_reduce(
            sq[:], t2[:].rearrange("p (j m) -> p j m", m=N_J),
            axis=mybir.AxisListType.X, op=Alu.add)
        rt = sv_pool.tile([16, CH_J], F32, tag="rt")
        nc.scalar.activation(rt[:], sq[:], Act.Sqrt, bias=eps_t[:])
        onep = sv_pool.tile([16, CH_J], F32, tag="onep")
        nc.vector.tensor_scalar_add(onep[:], sq[:], 1.0)
        den = sv_pool.tile([16, CH_J], F32, tag="den")
        nc.vector.tensor_mul(den[:], rt[:], onep[:])
        rec = sv_pool.tile([16, CH_J], F32, tag="rec")
        nc.vector.reciprocal(rec[:], den[:])
        fs = sv_pool.tile([16, CH_J], F32, tag="fs")
        nc.vector.tensor_mul(fs[:], sq[:], rec[:])
        vv = sv_pool.tile([16, JM], F32 if last else BF16, tag="vv")
        nc.vector.tensor_mul(
            vv[:].rearrange("p (j m) -> p j m", m=N_J),
            sf[:].rearrange("p (j m) -> p j m", m=N_J),
            fs[:].unsqueeze(-1).broadcast_to((16, CH_J, N_J)))
        return vv

    v_sb = allreduce_squash(0, s_sb, last=False)

    # ---------------- Phases C: routing passes r=1,2 ----------------
    for r in (1, 2):
        with tc.tile_pool(name=f"vr{r}", bufs=1) as vr_pool, \
             tc.tile_pool(name=f"scr{r}", bufs=2) as scr_pool, \
             tc.tile_pool(name=f"ce{r}", bufs=2) as ce_pool, \
             tc.tile_pool(name=f"cu{r}", bufs=2) as cu_pool, \
             tc.tile_pool(name=f"t1_{r}", bufs=2) as t1_pool, \
             tc.tile_pool(name=f"t2_{r}", bufs=2) as t2_pool, \
             tc.tile_pool(name=f"t3_{r}", bufs=2) as t3_pool, \
             tc.tile_pool(name=f"sm{r}", bufs=3) as sm_pool, \
             tc.tile_pool(name=f"vrps{r}", bufs=2, space="PSUM") as vr_ps_pool, \
             tc.tile_pool(name=f"sps{r}", bufs=1, space="PSUM") as s_ps_pool:
            # v_rep (128, JM) bf16: broadcast v over the 8 i-slots
            v_rep = vr_pool.tile([128, JM], BF16)
            for h in range(2):
                vp = vr_ps_pool.tile([128, 1024], F32)
                for q in range(2):
                    nc.tensor.matmul(
                        vp[:, q * 512:(q + 1) * 512], r8[:],
                        v_sb[:, h * 1024 + q * 512:h * 1024 + (q + 1) * 512],
                        start=True, stop=True)
                nc.scalar.activation(v_rep[:, h * 1024:(h + 1) * 1024],
                                     vp[:], Act.Copy)
            s_ps = s_ps_pool.tile([16, JM], F32)
            for t in range(N_T):
                ut = u_tiles[t]
                uv = scr_pool.tile([128, JM], BF16, tag="uv")
                nc.vector.tensor_mul(uv[:], ut[:], v_rep[:])
                # m-reduction as a binary tree of 2x-mode TT adds
                uv3 = uv[:].rearrange("p (j m) -> p j m", m=32)
                t1 = t1_pool.tile([128, 1024], BF16)
                t13 = t1[:].rearrange("p (j m) -> p j m", m=16)
                nc.vector.tensor_add(t13, uv3[:, :, 0:16], uv3[:, :, 16:32])
                t2_ = t2_pool.tile([128, 512], BF16)
                t23 = t2_[:].rearrange("p (j m) -> p j m", m=8)
                nc.vector.tensor_add(t23, t13[:, :, 0:8], t13[:, :, 8:16])
                t3_ = t3_pool.tile([128, 256], BF16)
                t33 = t3_[:].rearrange("p (j m) -> p j m", m=4)
                nc.vector.tensor_add(t33, t23[:, :, 0:4], t23[:, :, 4:8])
                t4 = sm_pool.tile([128, 128], BF16, tag="t4")
                t43 = t4[:].rearrange("p (j m) -> p j m", m=2)
                nc.vector.tensor_add(t43, t33[:, :, 0:2], t33[:, :, 2:4])
                if r == 1:
                    a1 = a1_pool.tile([128, CH_J], F32)
                    a1_tiles.append(a1)
                    nc.vector.tensor_add(
                        a1[:].rearrange("p (j m) -> p j m", m=1),
                        t43[:, :, 0:1], t43[:, :, 1:2])
                    logits = a1
                else:
                    a2 = sm_pool.tile([128, CH_J], F32, tag="a2")
                    nc.vector.tensor_add(
                        a2[:].rearrange("p (j m) -> p j m", m=1),
                        t43[:, :, 0:1], t43[:, :, 1:2])
                    lg = sm_pool.tile([128, CH_J], F32, tag="lg")
                    nc.vector.tensor_add(lg[:], a2[:], a1_tiles[t][:])
                    logits = lg
                expt = sm_pool.tile([128, CH_J], BF16, tag="expt")
                se = sm_pool.tile([128, 1], F32, tag="se")
                nc.scalar.activation(expt[:], logits[:], Act.Exp,
                                     scale=8.0, bias=mln[:],
                                     accum_out=se[:])
                rs = sm_pool.tile([128, 1], F32, tag="rs")
                nc.vector.reciprocal(rs[:], se[:])
                cexp = ce_pool.tile([128, JM], BF16, tag="cexp")
                nc.scalar.activation(
                    cexp[:].rearrange("p (j m) -> p j m", m=N_J),
                    expt[:].unsqueeze(-1).broadcast_to((128, CH_J, N_J)),
                    Act.Copy, scale=rs[:])
                cu = cu_pool.tile([128, JM], BF16, tag="cu")
                nc.vector.tensor_mul(cu[:], ut[:], cexp[:])
                for q in range(4):
                    nc.tensor.matmul(
                        s_ps[:, q * 512:(q + 1) * 512], s16[:],
                        cu[:, q * 512:(q + 1) * 512],
                        start=(t == 0), stop=(t == N_T - 1))
            s_sb2 = sv_pool.tile([16, JM], F32, tag="s_sb")
            nc.scalar.activation(s_sb2[:], s_ps[:], Act.Copy)
        v_sb = allreduce_squash(r, s_sb2, last=(r == 2))

    nc.sync.dma_start(v_d[:], v_sb[:])


def _host_inputs(inputs, w):
    """Build per-core input maps (host-side shard + block-diag pack)."""
    import ml_dtypes
    x = np.asarray(inputs, dtype=np.float32)
    w = np.asarray(w, dtype=np.float32)
    # 64.0 (not 1.0): compensates the exp's folded-in 1/CH_J normalization
    s16 = np.zeros((128, 16), dtype=np.float32)
    for i8 in range(8):
        for b in range(16):
            s16[i8 * 16 + b, b] = 64.0
    s16 = s16.astype(ml_dtypes.bfloat16)
    r8 = np.zeros((16, 128), dtype=np.float32)
    for i8 in range(8):
        for b in range(16):
            r8[b, i8 * 16 + b] = 1.0
    r8 = r8.astype(ml_dtypes.bfloat16)
    in_maps = []
    for k in range(N_CORES):
        i0 = k * I_LOC
        wk = w[i0:i0 + I_LOC].reshape(I_LOC * N_I, JM)
        wk = wk.astype(ml_dtypes.bfloat16)
        xk = x[:, i0:i0 + I_LOC, :]  # (B, 256, 16)
        xblk = np.zeros((N_T, 128, 128), dtype=np.float32)
        # xblk[t, i8*16+n, i8*16+b] = x[b, i0+8t+i8, n]
        xv = xk.transpose(1, 2, 0).reshape(N_T, 8, N_I, B)  # (t, i8, n, b)
        for i8 in range(8):
            xblk[:, i8 * 16:i8 * 16 + N_I, i8 * 16:i8 * 16 + B] = xv[:, i8]
        # xsum[t, i8*16+n, b] = x[b, i0+8t+i8, n]  (summed over i8 by matmul)
        xsum = xv.transpose(0, 1, 2, 3).reshape(N_T, 128, B).copy()
        in_maps.append({
            "xblk": xblk.astype(ml_dtypes.bfloat16),
            "xsum": xsum.astype(ml_dtypes.bfloat16),
            "w": wk, "s16": s16, "r8": r8})
    return in_maps


def kernel(inputs, w, _trace=False):
    key = "nc"
    if key not in _CACHE:
        _CACHE[key] = _build_program()
    nc = _CACHE[key]
    in_maps = _host_inputs(inputs, w)
    res = run_bass_kernel_spmd(nc, in_maps, list(range(N_CORES)),
                               trace=_trace)
    v = res.results[0]["v"].reshape(B, CH_J, N_J).astype(np.float32)
    if _trace:
        kernel._last = res
    return v
